# revision 9
# baseline (speedup 1.0000x reference)
"""CANet non-local attention block (sparse_attention) on 8 Trainium2 cores.

Math (per sample, reference.py):
    g     = maxpool2(conv1x1(x, g_w, g_b))        -> [CI, 2304]
    theta = conv1x1(x, theta_w, theta_b)          -> [CI, 9216]
    phi   = maxpool2(conv1x1(x, phi_w, phi_b))    -> [CI, 2304]
    f     = theta^T @ phi                         -> [9216, 2304]
    attn  = softmax(f, axis=-1)
    y     = attn @ g^T                            -> [9216, CI]
    out   = BN(conv1x1(y^T, W_w, W_b)) + x

Sharding: 8 cores = 4 samples x 2 query-halves. Each core computes phi/g
over its full sample (redundantly with its pair core) and theta/attention
for its 4608-query half.

Device-side design notes:
  - All matmuls in float32r (full fp32 data, 1 cycle/row at free-dim>=256).
  - Softmax without a max pass: exp(f - 20) is safe (|f| < ~60) and the
    normalizer s = sum_k e is obtained for free by augmenting g^T with a
    ones column in the second matmul (y_aug row 64 = s).
  - BN + W_b folded into W' = inv*W_w on host; the residual-side bias b'
    is folded into x' = x + b' with projection biases compensated.
  - Final normalize: r = 1/s (DVE fast reciprocal), broadcast across
    partitions with a K=1 PE matmul against a ones row.
"""

import sys
import types

if "/opt/trn_rl_repo" not in sys.path:
    sys.path.insert(0, "/opt/trn_rl_repo")

# antenv.axon_hooks is absent in this image, so trn_boot's NTFF hook install
# silently degrades. Provide the module and install the ctypes hook ourselves
# so run_bass_kernel_spmd(trace=True) can capture NTFF profiles.
try:
    import antenv

    if "antenv.axon_hooks" not in sys.modules:
        _m = types.ModuleType("antenv.axon_hooks")
        _hook_box = [None]

        def _set(h):
            _hook_box[0] = h

        def _get():
            return _hook_box[0]

        _m.set_axon_ntff_profile_hook = _set
        _m.get_axon_ntff_profile_hook = _get
        sys.modules["antenv.axon_hooks"] = _m
        antenv.axon_hooks = _m
        try:
            if "/root/.axon_site" not in sys.path:
                sys.path.insert(0, "/root/.axon_site")
            from trn_agent_boot.trn_boot import _ntff_profile_via_ctypes

            _hook_box[0] = _ntff_profile_via_ctypes("/opt/axon/libaxon_pjrt.so")
        except Exception:
            pass
except Exception:
    pass

import numpy as np

import concourse.bass as bass
import concourse.tile as tile
from concourse import mybir
from concourse.bass_utils import run_bass_kernel_spmd

F32 = mybir.dt.float32
F32R = mybir.dt.float32r

B, C, CI, H, W = 4, 128, 64, 96, 96
N = H * W            # 9216 queries per sample
NKV = (H // 2) * (W // 2)   # 2304 keys
NQH = N // 2         # 4608 queries per core
QB = 512             # query block
NB = NQH // QB       # 9 blocks per core
KCH = NKV // 128     # 18 kv chunks of 128
EXP_SHIFT = -20.0

_PROGRAM = None  # (nc, run-callable cache)


def _r(ap):
    return ap.bitcast(F32R)


def _split_multi_waits(nc, max_waits=1):
    """walrus codegen in this container only accepts one sync-wait command
    per instruction; hoist extras onto injected same-engine NoOps."""
    n_new = 0
    for f in nc.m.functions:
        for bb in f.blocks:
            new_list = []
            for ins in bb.instructions:
                si = ins.sync_info
                w = list(si.on_wait) if si and si.on_wait else []
                if len(w) > max_waits:
                    extras, keep = w[:-max_waits], w[-max_waits:]
                    for ew in extras:
                        nop = mybir.InstNoOp(
                            name=f"I-ws{nc.next_id()}", ins=[], outs=[]
                        )
                        nop.engine = ins.engine
                        nop.sync_info = mybir.SyncInfo(on_wait=[ew], on_update=[])
                        new_list.append(nop)
                        n_new += 1
                    si.on_wait = keep
                    ins.sync_info = si
                new_list.append(ins)
            bb.instructions[:] = new_list
    return n_new


def _build_program():
    nc = bass.Bass("TRN2", target_bir_lowering=False, debug=False, num_devices=8)

    xf = nc.dram_tensor("xf", [C, N], F32R, kind="ExternalInput")
    xh = nc.dram_tensor("xh", [C, NQH], F32R, kind="ExternalInput")
    wgp = nc.dram_tensor("wgp", [C, 128], F32R, kind="ExternalInput")
    wt = nc.dram_tensor("wt", [C, CI], F32R, kind="ExternalInput")
    wpc = nc.dram_tensor("wpc", [CI, C], F32R, kind="ExternalInput")
    bgp = nc.dram_tensor("bgp", [C, 1], F32, kind="ExternalInput")
    bt = nc.dram_tensor("bt", [CI, 1], F32, kind="ExternalInput")
    id64 = nc.dram_tensor("id64", [CI, CI], F32R, kind="ExternalInput")
    out = nc.dram_tensor("out", [C, NQH], F32, kind="ExternalOutput")

    with tile.TileContext(nc) as tc:
        with (
            tc.tile_pool(name="const", bufs=1) as const,
            tc.tile_pool(name="main", bufs=1) as main,
            tc.tile_pool(name="small", bufs=3) as small,
            tc.tile_pool(name="fps", bufs=2, space="PSUM") as fps,
            tc.tile_pool(name="sps", bufs=2, space="PSUM") as sps,
        ):
            wgp_sb = const.tile([C, 128], F32R)
            nc.sync.dma_start(wgp_sb, wgp[:, :])
            wt_sb = const.tile([C, CI], F32R)
            nc.sync.dma_start(wt_sb, wt[:, :])
            wpc_sb = const.tile([CI, C], F32R)
            nc.sync.dma_start(wpc_sb, wpc[:, :])
            bgp_sb = const.tile([C, 1], F32)
            nc.sync.dma_start(bgp_sb, bgp[:, :])
            bt_sb = const.tile([CI, 1], F32)
            nc.sync.dma_start(bt_sb, bt[:, :])
            id_sb = const.tile([CI, CI], F32R)
            nc.sync.dma_start(id_sb, id64[:, :])
            ones_sb = const.tile([1, 128], F32)
            nc.vector.memset(ones_sb, 1.0)
            shift_sb = const.tile([C, 1], F32)
            nc.vector.memset(shift_sb, EXP_SHIFT)

            xh_sb = main.tile([C, NQH], F32R)
            nc.sync.dma_start(xh_sb, xh[:, :])
            th_sb = main.tile([CI, NQH], F32R)
            P_sb = main.tile([C, 48, 48], F32R)     # pooled [g(0:64); phi(64:128)]
            phi0 = main.tile([CI, NKV], F32R)       # phi copy at base partition 0
            gt_sb = main.tile([C, KCH * (CI + 1)], F32R)  # g^T chunks + ones col
            y_all = main.tile([CI + 1, NQH], F32R)  # y rows 0..63, s in row 64

            # ---- conv phase (gp_full lives in a pool we close afterwards) ----
            with (
                tc.tile_pool(name="big", bufs=1) as big,
                tc.tile_pool(name="xs", bufs=4) as xs,
            ):
                gp_full = big.tile([C, N], F32R)
                for t in range(N // 1536):  # 6 groups of 3x512
                    ft = fps.tile([C, 1536], F32, tag="fp")
                    for u in range(3):
                        j = 3 * t + u
                        xt = xs.tile([C, QB], F32R)
                        nc.sync.dma_start(xt, xf[:, j * QB:(j + 1) * QB])
                        nc.tensor.matmul(
                            ft[:, u * QB:(u + 1) * QB],
                            lhsT=wgp_sb,
                            rhs=xt,
                            start=True,
                            stop=True,
                        )
                    nc.scalar.add(
                        gp_full[:, t * 1536:(t + 1) * 1536], ft, bgp_sb
                    )
                for j in range(NB):
                    tp = sps.tile([C, QB], F32, tag="sp")
                    nc.tensor.matmul(
                        tp[0:CI, :],
                        lhsT=wt_sb,
                        rhs=xh_sb[:, j * QB:(j + 1) * QB],
                        start=True,
                        stop=True,
                    )
                    nc.scalar.add(
                        th_sb[:, j * QB:(j + 1) * QB], tp[0:CI, :], bt_sb
                    )

                # 2x2 maxpool over the 96x96 spatial grid
                v = gp_full[:, :].rearrange(
                    "p (h a w b) -> p h a w b", h=48, a=2, w=48, b=2
                )
                m1 = big.tile([C, 48, 48], F32R)
                m2 = big.tile([C, 48, 48], F32R)
                nc.vector.tensor_max(m1, v[:, :, 0, :, 0], v[:, :, 0, :, 1])
                nc.vector.tensor_max(m2, v[:, :, 1, :, 0], v[:, :, 1, :, 1])
                nc.vector.tensor_max(P_sb, m1, m2)

            P_flat = P_sb[:, :, :].rearrange("p h w -> p (h w)")
            nc.vector.tensor_copy(phi0, P_flat[CI:C, :])

            # g^T chunks [128kv, 64ci] via PE transpose, plus ones column
            for j in range(KCH):
                tp = sps.tile([C, QB], F32R, tag="sp")
                nc.tensor.transpose(
                    tp[:, 0:CI], P_flat[0:CI, j * 128:(j + 1) * 128], id_sb
                )
                nc.vector.tensor_copy(
                    gt_sb[:, j * (CI + 1):j * (CI + 1) + CI], tp[:, 0:CI]
                )
            onesc_sb = const.tile([C, 1], F32)
            nc.vector.memset(onesc_sb, 1.0)
            for j in range(KCH):
                nc.vector.tensor_copy(
                    gt_sb[:, j * (CI + 1) + CI:(j + 1) * (CI + 1)], onesc_sb
                )

            # ---- attention phase, software-pipelined over q-blocks ----
            with tc.tile_pool(name="epool", bufs=2) as epool:

                def emit_mm1(e_t, b, groups):
                    for t in groups:
                        ft = fps.tile([C, 1536], F32, tag="fp")
                        for u in range(3):
                            j = 3 * t + u
                            nc.tensor.matmul(
                                ft[:, u * QB:(u + 1) * QB],
                                lhsT=phi0[:, j * 128:(j + 1) * 128],
                                rhs=th_sb[:, b * QB:(b + 1) * QB],
                                start=True,
                                stop=True,
                            )
                        nc.scalar.activation(
                            e_t[:, t * 1536:(t + 1) * 1536],
                            ft,
                            mybir.ActivationFunctionType.Exp,
                            bias=shift_sb,
                        )

                def emit_mm2(e_t, y_ps, chunks, first, last):
                    for j in chunks:
                        nc.tensor.matmul(
                            y_ps[0:CI + 1, :],
                            lhsT=gt_sb[:, j * (CI + 1):(j + 1) * (CI + 1)],
                            rhs=e_t[:, j * QB:(j + 1) * QB],
                            start=(j == first),
                            stop=(j == last),
                            skip_group_check=True,
                        )

                e_cur = epool.tile([C, N], F32R, tag="e")
                emit_mm1(e_cur, 0, range(6))
                for b in range(NB):
                    y_ps = sps.tile([C, QB], F32, tag="sp")
                    if b + 1 < NB:
                        # interleave next block's QK^T/exp with this block's AV
                        e_nxt = epool.tile([C, N], F32R, tag="e")
                        for t in range(6):
                            emit_mm1(e_nxt, b + 1, [t])
                            emit_mm2(e_cur, y_ps, range(3 * t, 3 * t + 3), 0, 17)
                    else:
                        e_nxt = None
                        emit_mm2(e_cur, y_ps, range(KCH), 0, 17)

                    nc.vector.tensor_copy(
                        y_all[:, b * QB:(b + 1) * QB], y_ps[0:CI + 1, :]
                    )
                    r_t = small.tile([1, QB], F32)
                    nc.vector.reciprocal(
                        r_t, y_all[CI:CI + 1, b * QB:(b + 1) * QB].bitcast(F32)
                    )
                    br_ps = sps.tile([C, QB], F32, tag="sp")
                    nc.tensor.matmul(
                        br_ps, lhsT=ones_sb, rhs=r_t, start=True, stop=True
                    )
                    br_sb = small.tile([C, QB], F32)
                    nc.vector.tensor_copy(br_sb, br_ps)
                    z_ps = sps.tile([C, QB], F32, tag="sp")
                    nc.tensor.matmul(
                        z_ps,
                        lhsT=wpc_sb,
                        rhs=y_all[0:CI, b * QB:(b + 1) * QB],
                        start=True,
                        stop=True,
                    )
                    t1 = small.tile([C, QB], F32)
                    nc.vector.tensor_mul(t1, z_ps, br_sb)
                    ot = small.tile([C, QB], F32)
                    nc.vector.tensor_add(ot, t1, xh_sb[:, b * QB:(b + 1) * QB].bitcast(F32))
                    nc.sync.dma_start(out[:, b * QB:(b + 1) * QB], ot)
                    e_cur = e_nxt

    _split_multi_waits(nc)
    return nc


def _get_program():
    global _PROGRAM
    if _PROGRAM is None:
        _PROGRAM = _build_program()
    return _PROGRAM


def _host_prep(x, g_w, g_b, theta_w, theta_b, phi_w, phi_b, W_w, W_b,
               bn_gamma, bn_beta, bn_mean, bn_var):
    f32 = np.float32
    inv = (bn_gamma / np.sqrt(bn_var + 1e-5)).astype(f32)
    bprime = (W_b * inv + bn_beta - bn_mean * inv).astype(f32)
    xp = (x + bprime[None, :, None, None]).astype(f32)

    wgp = np.ascontiguousarray(np.concatenate([g_w, phi_w], 0).T, dtype=f32)
    wt = np.ascontiguousarray(theta_w.T, dtype=f32)
    wpc = np.ascontiguousarray((W_w * inv[:, None]).T, dtype=f32)
    bg = (g_b - g_w @ bprime).astype(f32)
    bp = (phi_b - phi_w @ bprime).astype(f32)
    bgp = np.concatenate([bg, bp]).reshape(C, 1).astype(f32)
    btv = (theta_b - theta_w @ bprime).reshape(CI, 1).astype(f32)
    id64 = np.eye(CI, dtype=f32)

    in_maps = []
    for core in range(8):
        s, h = core // 2, core % 2
        xs_full = np.ascontiguousarray(xp[s].reshape(C, N))
        xs_half = np.ascontiguousarray(
            xp[s, :, 48 * h:48 * (h + 1), :].reshape(C, NQH)
        )
        in_maps.append(
            {
                "xf": xs_full,
                "xh": xs_half,
                "wgp": wgp,
                "wt": wt,
                "wpc": wpc,
                "bgp": bgp,
                "bt": btv,
                "id64": id64,
            }
        )
    return in_maps


def run_cores(in_maps, trace=False):
    nc = _get_program()
    return run_bass_kernel_spmd(nc, in_maps, list(range(8)), trace=trace)


def kernel(**inputs) -> np.ndarray:
    in_maps = _host_prep(**inputs)
    res = run_cores(in_maps)
    out = np.empty((B, C, H, W), dtype=np.float32)
    for core in range(8):
        s, h = core // 2, core % 2
        out[s, :, 48 * h:48 * (h + 1), :] = res.results[core]["out"].reshape(
            C, 48, W
        )
    return out


# revision 10
# speedup vs baseline: 1.0922x; 1.0922x over previous
"""CANet non-local attention block (sparse_attention) on 8 Trainium2 cores.

Math (per sample, reference.py):
    g     = maxpool2(conv1x1(x, g_w, g_b))        -> [CI, 2304]
    theta = conv1x1(x, theta_w, theta_b)          -> [CI, 9216]
    phi   = maxpool2(conv1x1(x, phi_w, phi_b))    -> [CI, 2304]
    f     = theta^T @ phi                         -> [9216, 2304]
    attn  = softmax(f, axis=-1)
    y     = attn @ g^T                            -> [9216, CI]
    out   = BN(conv1x1(y^T, W_w, W_b)) + x

Sharding: 8 cores = 4 samples x 2 query-halves. Each core computes phi/g
over its full sample (redundantly with its pair core) and theta/attention
for its 4608-query half.

Device-side design notes:
  - All matmuls in float32r (full fp32 data, 1 cycle/row at free-dim>=256).
  - Softmax without a max pass: exp(f - 20) is safe (|f| < ~60) and the
    normalizer s = sum_k e is obtained for free by augmenting g^T with a
    ones column in the second matmul (y_aug row 64 = s).
  - BN + W_b folded into W' = inv*W_w on host; the residual-side bias b'
    is folded into x' = x + b' with projection biases compensated.
  - Final normalize: r = 1/s (DVE fast reciprocal), broadcast across
    partitions with a K=1 PE matmul against a ones row.
"""

import sys
import types

if "/opt/trn_rl_repo" not in sys.path:
    sys.path.insert(0, "/opt/trn_rl_repo")

# antenv.axon_hooks is absent in this image, so trn_boot's NTFF hook install
# silently degrades. Provide the module and install the ctypes hook ourselves
# so run_bass_kernel_spmd(trace=True) can capture NTFF profiles.
try:
    import antenv

    if "antenv.axon_hooks" not in sys.modules:
        _m = types.ModuleType("antenv.axon_hooks")
        _hook_box = [None]

        def _set(h):
            _hook_box[0] = h

        def _get():
            return _hook_box[0]

        _m.set_axon_ntff_profile_hook = _set
        _m.get_axon_ntff_profile_hook = _get
        sys.modules["antenv.axon_hooks"] = _m
        antenv.axon_hooks = _m
        try:
            if "/root/.axon_site" not in sys.path:
                sys.path.insert(0, "/root/.axon_site")
            from trn_agent_boot.trn_boot import _ntff_profile_via_ctypes

            _hook_box[0] = _ntff_profile_via_ctypes("/opt/axon/libaxon_pjrt.so")
        except Exception:
            pass
except Exception:
    pass

import numpy as np

import concourse.bass as bass
import concourse.tile as tile
from concourse import mybir
from concourse.bass_utils import run_bass_kernel_spmd

F32 = mybir.dt.float32
F32R = mybir.dt.float32r
F16 = mybir.dt.float16
BF16 = mybir.dt.bfloat16

B, C, CI, H, W = 4, 128, 64, 96, 96
N = H * W            # 9216 queries per sample
NKV = (H // 2) * (W // 2)   # 2304 keys
NQH = N // 2         # 4608 queries per core
QB = 512             # query block
NB = NQH // QB       # 9 blocks per core
KCH = NKV // 128     # 18 kv chunks of 128
EXP_SHIFT = -20.0

_PROGRAM = None  # (nc, run-callable cache)


def _r(ap):
    return ap.bitcast(F32R)


def _split_multi_waits(nc, max_waits=1):
    """walrus codegen in this container only accepts one sync-wait command
    per instruction; hoist extras onto injected same-engine NoOps."""
    n_new = 0
    for f in nc.m.functions:
        for bb in f.blocks:
            new_list = []
            for ins in bb.instructions:
                si = ins.sync_info
                w = list(si.on_wait) if si and si.on_wait else []
                if len(w) > max_waits:
                    extras, keep = w[:-max_waits], w[-max_waits:]
                    for ew in extras:
                        nop = mybir.InstNoOp(
                            name=f"I-ws{nc.next_id()}", ins=[], outs=[]
                        )
                        nop.engine = ins.engine
                        nop.sync_info = mybir.SyncInfo(on_wait=[ew], on_update=[])
                        new_list.append(nop)
                        n_new += 1
                    si.on_wait = keep
                    ins.sync_info = si
                new_list.append(ins)
            bb.instructions[:] = new_list
    return n_new


def _build_program():
    nc = bass.Bass("TRN2", target_bir_lowering=False, debug=False, num_devices=8)

    xf = nc.dram_tensor("xf", [C, N], F16, kind="ExternalInput")
    xh = nc.dram_tensor("xh", [C, NQH], F32, kind="ExternalInput")
    xh16 = nc.dram_tensor("xh16", [C, NQH], F16, kind="ExternalInput")
    wgp = nc.dram_tensor("wgp", [C, 128], F16, kind="ExternalInput")
    wt = nc.dram_tensor("wt", [C, CI], F16, kind="ExternalInput")
    wpc = nc.dram_tensor("wpc", [CI, C], BF16, kind="ExternalInput")
    bgp = nc.dram_tensor("bgp", [C, 1], F32, kind="ExternalInput")
    bt = nc.dram_tensor("bt", [CI, 1], F32, kind="ExternalInput")
    id64 = nc.dram_tensor("id64", [CI, CI], F16, kind="ExternalInput")
    out = nc.dram_tensor("out", [C, NQH], F32, kind="ExternalOutput")

    with tile.TileContext(nc) as tc:
        with (
            tc.tile_pool(name="const", bufs=1) as const,
            tc.tile_pool(name="main", bufs=1) as main,
            tc.tile_pool(name="small", bufs=3) as small,
            tc.tile_pool(name="fps", bufs=2, space="PSUM") as fps,
            tc.tile_pool(name="sps", bufs=2, space="PSUM") as sps,
        ):
            wgp_sb = const.tile([C, 128], F16)
            nc.sync.dma_start(wgp_sb, wgp[:, :])
            wt_sb = const.tile([C, CI], F16)
            nc.sync.dma_start(wt_sb, wt[:, :])
            wpc_sb = const.tile([CI, C], BF16)
            nc.sync.dma_start(wpc_sb, wpc[:, :])
            bgp_sb = const.tile([C, 1], F32)
            nc.sync.dma_start(bgp_sb, bgp[:, :])
            bt_sb = const.tile([CI, 1], F32)
            nc.sync.dma_start(bt_sb, bt[:, :])
            id_sb = const.tile([CI, CI], F16)
            nc.sync.dma_start(id_sb, id64[:, :])
            ones_sb = const.tile([1, 128], F32)
            nc.vector.memset(ones_sb, 1.0)
            shift_sb = const.tile([C, 1], F32)
            nc.vector.memset(shift_sb, EXP_SHIFT)

            xh_sb = main.tile([C, NQH], F32)
            nc.sync.dma_start(xh_sb, xh[:, :])
            xh16_sb = main.tile([C, NQH], F16)
            nc.sync.dma_start(xh16_sb, xh16[:, :])
            th_sb = main.tile([CI, NQH], F16)
            P_sb = main.tile([C, 48, 48], F16)     # pooled [g(0:64); phi(64:128)]
            phi0 = main.tile([CI, NKV], F16)       # phi copy at base partition 0
            gt_sb = main.tile([C, KCH * (CI + 1)], BF16)  # g^T chunks + ones col
            y_all = main.tile([CI + 1, NQH], BF16)  # y rows 0..63, s in row 64

            # ---- conv phase (gp_full lives in a pool we close afterwards) ----
            with (
                tc.tile_pool(name="big", bufs=1) as big,
                tc.tile_pool(name="xs", bufs=4) as xs,
            ):
                gp_full = big.tile([C, N], F16)
                for t in range(N // 1536):  # 6 groups of 3x512
                    ft = fps.tile([C, 1536], F32, tag="fp")
                    for u in range(3):
                        j = 3 * t + u
                        xt = xs.tile([C, QB], F16)
                        nc.sync.dma_start(xt, xf[:, j * QB:(j + 1) * QB])
                        nc.tensor.matmul(
                            ft[:, u * QB:(u + 1) * QB],
                            lhsT=wgp_sb,
                            rhs=xt,
                            start=True,
                            stop=True,
                        )
                    nc.scalar.add(
                        gp_full[:, t * 1536:(t + 1) * 1536], ft, bgp_sb
                    )
                for j in range(NB):
                    tp = sps.tile([C, QB], F32, tag="sp")
                    nc.tensor.matmul(
                        tp[0:CI, :],
                        lhsT=wt_sb,
                        rhs=xh16_sb[:, j * QB:(j + 1) * QB],
                        start=True,
                        stop=True,
                    )
                    nc.scalar.add(
                        th_sb[:, j * QB:(j + 1) * QB], tp[0:CI, :], bt_sb
                    )

                # 2x2 maxpool over the 96x96 spatial grid
                v = gp_full[:, :].rearrange(
                    "p (h a w b) -> p h a w b", h=48, a=2, w=48, b=2
                )
                m1 = big.tile([C, 48, 48], F16)
                m2 = big.tile([C, 48, 48], F16)
                nc.vector.tensor_max(m1, v[:, :, 0, :, 0], v[:, :, 0, :, 1])
                nc.vector.tensor_max(m2, v[:, :, 1, :, 0], v[:, :, 1, :, 1])
                nc.vector.tensor_max(P_sb, m1, m2)

            P_flat = P_sb[:, :, :].rearrange("p h w -> p (h w)")
            nc.vector.tensor_copy(phi0, P_flat[CI:C, :])

            # g^T chunks [128kv, 64ci] via PE transpose, plus ones column
            for j in range(KCH):
                tp = sps.tile([C, QB], F16, tag="sp")
                nc.tensor.transpose(
                    tp[:, 0:CI], P_flat[0:CI, j * 128:(j + 1) * 128], id_sb
                )
                nc.vector.tensor_copy(
                    gt_sb[:, j * (CI + 1):j * (CI + 1) + CI], tp[:, 0:CI]
                )
            onesc_sb = const.tile([C, 1], F32)
            nc.vector.memset(onesc_sb, 1.0)
            for j in range(KCH):
                nc.vector.tensor_copy(
                    gt_sb[:, j * (CI + 1) + CI:(j + 1) * (CI + 1)], onesc_sb
                )

            # ---- attention phase, software-pipelined over q-blocks ----
            with tc.tile_pool(name="epool", bufs=2) as epool:

                def emit_mm1(e_t, b, groups):
                    for t in groups:
                        ft = fps.tile([C, 1536], F32, tag="fp")
                        for u in range(3):
                            j = 3 * t + u
                            nc.tensor.matmul(
                                ft[:, u * QB:(u + 1) * QB],
                                lhsT=phi0[:, j * 128:(j + 1) * 128],
                                rhs=th_sb[:, b * QB:(b + 1) * QB],
                                start=True,
                                stop=True,
                            )
                        nc.scalar.activation(
                            e_t[:, t * 1536:(t + 1) * 1536],
                            ft,
                            mybir.ActivationFunctionType.Exp,
                            bias=shift_sb,
                        )

                def emit_mm2(e_t, y_ps, chunks, first, last):
                    for j in chunks:
                        nc.tensor.matmul(
                            y_ps[0:CI + 1, :],
                            lhsT=gt_sb[:, j * (CI + 1):(j + 1) * (CI + 1)],
                            rhs=e_t[:, j * QB:(j + 1) * QB],
                            start=(j == first),
                            stop=(j == last),
                            skip_group_check=True,
                        )

                e_cur = epool.tile([C, N], BF16, tag="e")
                emit_mm1(e_cur, 0, range(6))
                for b in range(NB):
                    y_ps = sps.tile([C, QB], F32, tag="sp")
                    if b + 1 < NB:
                        # interleave next block's QK^T/exp with this block's AV
                        e_nxt = epool.tile([C, N], BF16, tag="e")
                        for t in range(6):
                            emit_mm1(e_nxt, b + 1, [t])
                            emit_mm2(e_cur, y_ps, range(3 * t, 3 * t + 3), 0, 17)
                    else:
                        e_nxt = None
                        emit_mm2(e_cur, y_ps, range(KCH), 0, 17)

                    nc.vector.tensor_copy(
                        y_all[0:CI, b * QB:(b + 1) * QB], y_ps[0:CI, :]
                    )
                    s_t = small.tile([1, QB], F32)
                    nc.vector.tensor_copy(s_t, y_ps[CI:CI + 1, :])
                    r_t = small.tile([1, QB], F32)
                    nc.vector.reciprocal(r_t, s_t)
                    br_ps = sps.tile([C, QB], F32, tag="sp")
                    nc.tensor.matmul(
                        br_ps, lhsT=ones_sb, rhs=r_t, start=True, stop=True
                    )
                    br_sb = small.tile([C, QB], F32)
                    nc.vector.tensor_copy(br_sb, br_ps)
                    z_ps = sps.tile([C, QB], F32, tag="sp")
                    nc.tensor.matmul(
                        z_ps,
                        lhsT=wpc_sb,
                        rhs=y_all[0:CI, b * QB:(b + 1) * QB],
                        start=True,
                        stop=True,
                    )
                    t1 = small.tile([C, QB], F32)
                    nc.vector.tensor_mul(t1, z_ps, br_sb)
                    ot = small.tile([C, QB], F32)
                    nc.vector.tensor_add(ot, t1, xh_sb[:, b * QB:(b + 1) * QB])
                    nc.sync.dma_start(out[:, b * QB:(b + 1) * QB], ot)
                    e_cur = e_nxt

    _split_multi_waits(nc)
    return nc


def _get_program():
    global _PROGRAM
    if _PROGRAM is None:
        _PROGRAM = _build_program()
    return _PROGRAM


def _host_prep(x, g_w, g_b, theta_w, theta_b, phi_w, phi_b, W_w, W_b,
               bn_gamma, bn_beta, bn_mean, bn_var):
    f32 = np.float32
    inv = (bn_gamma / np.sqrt(bn_var + 1e-5)).astype(f32)
    bprime = (W_b * inv + bn_beta - bn_mean * inv).astype(f32)
    xp = (x + bprime[None, :, None, None]).astype(f32)

    import ml_dtypes

    wgp = np.ascontiguousarray(np.concatenate([g_w, phi_w], 0).T.astype(np.float16))
    wt = np.ascontiguousarray(theta_w.T.astype(np.float16))
    wpc = np.ascontiguousarray((W_w * inv[:, None]).T.astype(ml_dtypes.bfloat16))
    bg = (g_b - g_w @ bprime).astype(f32)
    bp = (phi_b - phi_w @ bprime).astype(f32)
    bgp = np.concatenate([bg, bp]).reshape(C, 1).astype(f32)
    btv = (theta_b - theta_w @ bprime).reshape(CI, 1).astype(f32)
    id64 = np.eye(CI, dtype=np.float16)

    in_maps = []
    for core in range(8):
        s, h = core // 2, core % 2
        xs_full = np.ascontiguousarray(xp[s].reshape(C, N).astype(np.float16))
        xs_half = np.ascontiguousarray(
            xp[s, :, 48 * h:48 * (h + 1), :].reshape(C, NQH)
        )
        in_maps.append(
            {
                "xf": xs_full,
                "xh": xs_half,
                "xh16": xs_half.astype(np.float16),
                "wgp": wgp,
                "wt": wt,
                "wpc": wpc,
                "bgp": bgp,
                "bt": btv,
                "id64": id64,
            }
        )
    return in_maps


def run_cores(in_maps, trace=False):
    nc = _get_program()
    return run_bass_kernel_spmd(nc, in_maps, list(range(8)), trace=trace)


def kernel(**inputs) -> np.ndarray:
    in_maps = _host_prep(**inputs)
    res = run_cores(in_maps)
    out = np.empty((B, C, H, W), dtype=np.float32)
    for core in range(8):
        s, h = core // 2, core % 2
        out[s, :, 48 * h:48 * (h + 1), :] = res.results[core]["out"].reshape(
            C, 48, W
        )
    return out


# revision 11
# speedup vs baseline: 1.2792x; 1.1712x over previous
"""CANet non-local attention block (sparse_attention) on 8 Trainium2 cores.

Math (per sample, reference.py):
    g     = maxpool2(conv1x1(x, g_w, g_b))        -> [CI, 2304]
    theta = conv1x1(x, theta_w, theta_b)          -> [CI, 9216]
    phi   = maxpool2(conv1x1(x, phi_w, phi_b))    -> [CI, 2304]
    f     = theta^T @ phi                         -> [9216, 2304]
    attn  = softmax(f, axis=-1)
    y     = attn @ g^T                            -> [9216, CI]
    out   = BN(conv1x1(y^T, W_w, W_b)) + x

Sharding: 8 cores = 4 samples x 2 query-halves. Each core computes phi/g
over its full sample (redundantly with its pair core) and theta/attention
for its 4608-query half.

Device-side design notes:
  - All matmuls in float32r (full fp32 data, 1 cycle/row at free-dim>=256).
  - Softmax without a max pass: exp(f - 20) is safe (|f| < ~60) and the
    normalizer s = sum_k e is obtained for free by augmenting g^T with a
    ones column in the second matmul (y_aug row 64 = s).
  - BN + W_b folded into W' = inv*W_w on host; the residual-side bias b'
    is folded into x' = x + b' with projection biases compensated.
  - Final normalize: r = 1/s (DVE fast reciprocal), broadcast across
    partitions with a K=1 PE matmul against a ones row.
"""

import sys
import types

if "/opt/trn_rl_repo" not in sys.path:
    sys.path.insert(0, "/opt/trn_rl_repo")

# antenv.axon_hooks is absent in this image, so trn_boot's NTFF hook install
# silently degrades. Provide the module and install the ctypes hook ourselves
# so run_bass_kernel_spmd(trace=True) can capture NTFF profiles.
try:
    import antenv

    if "antenv.axon_hooks" not in sys.modules:
        _m = types.ModuleType("antenv.axon_hooks")
        _hook_box = [None]

        def _set(h):
            _hook_box[0] = h

        def _get():
            return _hook_box[0]

        _m.set_axon_ntff_profile_hook = _set
        _m.get_axon_ntff_profile_hook = _get
        sys.modules["antenv.axon_hooks"] = _m
        antenv.axon_hooks = _m
        try:
            if "/root/.axon_site" not in sys.path:
                sys.path.insert(0, "/root/.axon_site")
            from trn_agent_boot.trn_boot import _ntff_profile_via_ctypes

            _hook_box[0] = _ntff_profile_via_ctypes("/opt/axon/libaxon_pjrt.so")
        except Exception:
            pass
except Exception:
    pass

import numpy as np

import concourse.bass as bass
import concourse.tile as tile
from concourse import mybir
from concourse.bass_utils import run_bass_kernel_spmd

F32 = mybir.dt.float32
F32R = mybir.dt.float32r
F16 = mybir.dt.float16
BF16 = mybir.dt.bfloat16

B, C, CI, H, W = 4, 128, 64, 96, 96
N = H * W            # 9216 queries per sample
NKV = (H // 2) * (W // 2)   # 2304 keys
NQH = N // 2         # 4608 queries per core
QB = 512             # query block
NB = NQH // QB       # 9 blocks per core
KCH = NKV // 128     # 18 kv chunks of 128
EXP_SHIFT = -20.0

_PROGRAM = None  # (nc, run-callable cache)


def _r(ap):
    return ap.bitcast(F32R)


def _split_multi_waits(nc, max_waits=1):
    """walrus codegen in this container only accepts one sync-wait command
    per instruction; hoist extras onto injected same-engine NoOps."""
    n_new = 0
    for f in nc.m.functions:
        for bb in f.blocks:
            new_list = []
            for ins in bb.instructions:
                si = ins.sync_info
                w = list(si.on_wait) if si and si.on_wait else []
                if len(w) > max_waits:
                    extras, keep = w[:-max_waits], w[-max_waits:]
                    for ew in extras:
                        nop = mybir.InstNoOp(
                            name=f"I-ws{nc.next_id()}", ins=[], outs=[]
                        )
                        nop.engine = ins.engine
                        nop.sync_info = mybir.SyncInfo(on_wait=[ew], on_update=[])
                        new_list.append(nop)
                        n_new += 1
                    si.on_wait = keep
                    ins.sync_info = si
                new_list.append(ins)
            bb.instructions[:] = new_list
    return n_new


def _build_program():
    nc = bass.Bass("TRN2", target_bir_lowering=False, debug=False, num_devices=8)

    xf = nc.dram_tensor("xf", [C, N], F16, kind="ExternalInput")
    xh = nc.dram_tensor("xh", [C, NQH], F32, kind="ExternalInput")
    xh16 = nc.dram_tensor("xh16", [C, NQH], F16, kind="ExternalInput")
    wgp = nc.dram_tensor("wgp", [C, 128], F16, kind="ExternalInput")
    wt = nc.dram_tensor("wt", [C, CI], F16, kind="ExternalInput")
    wpc = nc.dram_tensor("wpc", [CI, C], BF16, kind="ExternalInput")
    bgp = nc.dram_tensor("bgp", [C, 1], F32, kind="ExternalInput")
    bt = nc.dram_tensor("bt", [CI, 1], F32, kind="ExternalInput")
    id64 = nc.dram_tensor("id64", [CI, CI], F16, kind="ExternalInput")
    out = nc.dram_tensor("out", [C, NQH], F32, kind="ExternalOutput")

    with tile.TileContext(nc) as tc:
        with (
            tc.tile_pool(name="const", bufs=1) as const,
            tc.tile_pool(name="main", bufs=1) as main,
            tc.tile_pool(name="small", bufs=3) as small,
            tc.tile_pool(name="fps", bufs=2, space="PSUM") as fps,
            tc.tile_pool(name="sps", bufs=2, space="PSUM") as sps,
        ):
            wgp_sb = const.tile([C, 128], F16)
            nc.sync.dma_start(wgp_sb, wgp[:, :])
            wt_sb = const.tile([C, CI], F16)
            nc.sync.dma_start(wt_sb, wt[:, :])
            wpc_sb = const.tile([CI, C], BF16)
            nc.sync.dma_start(wpc_sb, wpc[:, :])
            bgp_sb = const.tile([C, 1], F32)
            nc.sync.dma_start(bgp_sb, bgp[:, :])
            bt_sb = const.tile([CI, 1], F32)
            nc.sync.dma_start(bt_sb, bt[:, :])
            id_sb = const.tile([CI, CI], F16)
            nc.sync.dma_start(id_sb, id64[:, :])
            ones_sb = const.tile([1, 128], F32)
            nc.vector.memset(ones_sb, 1.0)
            shift_sb = const.tile([C, 1], F32)
            nc.vector.memset(shift_sb, EXP_SHIFT)

            xh_sb = main.tile([C, NQH], F32)
            nc.sync.dma_start(xh_sb, xh[:, :])
            xh16_sb = main.tile([C, NQH], F16)
            nc.sync.dma_start(xh16_sb, xh16[:, :])
            th_sb = main.tile([C, NQH], F16)
            P_sb = main.tile([C, 48, 48], F16)     # pooled [g(0:64); phi(64:128)]
            phi0 = main.tile([CI, NKV], F16)       # phi copy at base partition 0
            gt_sb = main.tile([C, KCH * (CI + 1)], BF16)  # g^T chunks + ones col
            y_all = main.tile([CI + 1, NQH], BF16)  # y rows 0..63, s in row 64

            # ---- conv phase (gp_full lives in a pool we close afterwards) ----
            with (
                tc.tile_pool(name="big", bufs=1) as big,
                tc.tile_pool(name="xs", bufs=4) as xs,
            ):
                gp_full = big.tile([C, N], F16)
                for t in range(N // 1536):  # 6 groups of 3x512
                    ft = fps.tile([C, 1536], F32, tag="fp")
                    for u in range(3):
                        j = 3 * t + u
                        xt = xs.tile([C, QB], F16)
                        nc.sync.dma_start(xt, xf[:, j * QB:(j + 1) * QB])
                        nc.tensor.matmul(
                            ft[:, u * QB:(u + 1) * QB],
                            lhsT=wgp_sb,
                            rhs=xt,
                            start=True,
                            stop=True,
                        )
                    nc.scalar.add(
                        gp_full[:, t * 1536:(t + 1) * 1536], ft, bgp_sb
                    )
                for j in range(NB):
                    tp = sps.tile([C, QB], F32, tag="sp")
                    nc.tensor.matmul(
                        tp[0:CI, :],
                        lhsT=wt_sb,
                        rhs=xh16_sb[:, j * QB:(j + 1) * QB],
                        start=True,
                        stop=True,
                    )
                    nc.scalar.add(
                        th_sb[0:CI, j * QB:(j + 1) * QB], tp[0:CI, :], bt_sb
                    )
                    nc.scalar.add(
                        th_sb[CI:C, j * QB:(j + 1) * QB], tp[0:CI, :], bt_sb
                    )

                # 2x2 maxpool over the 96x96 spatial grid
                v = gp_full[:, :].rearrange(
                    "p (h a w b) -> p h a w b", h=48, a=2, w=48, b=2
                )
                m1 = big.tile([C, 48, 48], F16)
                m2 = big.tile([C, 48, 48], F16)
                nc.vector.tensor_max(m1, v[:, :, 0, :, 0], v[:, :, 0, :, 1])
                nc.vector.tensor_max(m2, v[:, :, 1, :, 0], v[:, :, 1, :, 1])
                nc.vector.tensor_max(P_sb, m1, m2)

            P_flat = P_sb[:, :, :].rearrange("p h w -> p (h w)")
            nc.vector.tensor_copy(phi0, P_flat[CI:C, :])

            # g^T chunks [128kv, 64ci] via PE transpose, plus ones column
            for j in range(KCH):
                tp = sps.tile([C, QB], F16, tag="sp")
                nc.tensor.transpose(
                    tp[:, 0:CI], P_flat[0:CI, j * 128:(j + 1) * 128], id_sb
                )
                nc.vector.tensor_copy(
                    gt_sb[:, j * (CI + 1):j * (CI + 1) + CI], tp[:, 0:CI]
                )
            onesc_sb = const.tile([C, 1], F32)
            nc.vector.memset(onesc_sb, 1.0)
            for j in range(KCH):
                nc.vector.tensor_copy(
                    gt_sb[:, j * (CI + 1) + CI:(j + 1) * (CI + 1)], onesc_sb
                )

            # ---- attention phase, software-pipelined over q-blocks ----
            with tc.tile_pool(name="epool", bufs=2) as epool:

                def emit_mm1(e_t, b, groups):
                    for t in groups:
                        ft = fps.tile([C, 1536], F32, tag="fp")
                        for u in range(3):
                            j = 3 * t + u
                            if j % 2 == 0:
                                lhsT = phi0[:, j * 128:(j + 1) * 128]
                                rhs = th_sb[0:CI, b * QB:(b + 1) * QB]
                                pos = (0, 0)
                            else:
                                lhsT = P_flat[CI:C, j * 128:(j + 1) * 128]
                                rhs = th_sb[CI:C, b * QB:(b + 1) * QB]
                                pos = (64, 0)
                            nc.tensor.matmul(
                                ft[:, u * QB:(u + 1) * QB],
                                lhsT=lhsT,
                                rhs=rhs,
                                start=True,
                                stop=True,
                                tile_position=pos,
                            )
                        nc.scalar.activation(
                            e_t[:, t * 1536:(t + 1) * 1536],
                            ft,
                            mybir.ActivationFunctionType.Exp,
                            bias=shift_sb,
                        )

                def emit_mm2(e_t, y_ps, chunks, first, last):
                    for j in chunks:
                        nc.tensor.matmul(
                            y_ps[0:CI + 1, :],
                            lhsT=gt_sb[:, j * (CI + 1):(j + 1) * (CI + 1)],
                            rhs=e_t[:, j * QB:(j + 1) * QB],
                            start=(j == first),
                            stop=(j == last),
                            skip_group_check=True,
                        )

                e_cur = epool.tile([C, N], BF16, tag="e")
                emit_mm1(e_cur, 0, range(6))
                for b in range(NB):
                    y_ps = sps.tile([C, QB], F32, tag="sp")
                    if b + 1 < NB:
                        # interleave next block's QK^T/exp with this block's AV
                        e_nxt = epool.tile([C, N], BF16, tag="e")
                        for t in range(6):
                            emit_mm2(e_cur, y_ps, range(3 * t, 3 * t + 3), 0, 17)
                            emit_mm1(e_nxt, b + 1, [t])
                    else:
                        e_nxt = None
                        emit_mm2(e_cur, y_ps, range(KCH), 0, 17)

                    nc.vector.tensor_copy(
                        y_all[0:CI, b * QB:(b + 1) * QB], y_ps[0:CI, :]
                    )
                    s_t = small.tile([1, QB], F32)
                    nc.vector.tensor_copy(s_t, y_ps[CI:CI + 1, :])
                    r_t = small.tile([1, QB], F32)
                    nc.vector.reciprocal(r_t, s_t)
                    br_ps = sps.tile([C, QB], F32, tag="sp")
                    nc.tensor.matmul(
                        br_ps, lhsT=ones_sb, rhs=r_t, start=True, stop=True
                    )
                    br_sb = small.tile([C, QB], F32)
                    nc.vector.tensor_copy(br_sb, br_ps)
                    z_ps = sps.tile([C, QB], F32, tag="sp")
                    nc.tensor.matmul(
                        z_ps,
                        lhsT=wpc_sb,
                        rhs=y_all[0:CI, b * QB:(b + 1) * QB],
                        start=True,
                        stop=True,
                    )
                    t1 = small.tile([C, QB], F32)
                    nc.vector.tensor_mul(t1, z_ps, br_sb)
                    ot = small.tile([C, QB], F32)
                    nc.vector.tensor_add(ot, t1, xh_sb[:, b * QB:(b + 1) * QB])
                    nc.sync.dma_start(out[:, b * QB:(b + 1) * QB], ot)
                    e_cur = e_nxt

    _split_multi_waits(nc)
    return nc


def _get_program():
    global _PROGRAM
    if _PROGRAM is None:
        _PROGRAM = _build_program()
    return _PROGRAM


def _host_prep(x, g_w, g_b, theta_w, theta_b, phi_w, phi_b, W_w, W_b,
               bn_gamma, bn_beta, bn_mean, bn_var):
    f32 = np.float32
    inv = (bn_gamma / np.sqrt(bn_var + 1e-5)).astype(f32)
    bprime = (W_b * inv + bn_beta - bn_mean * inv).astype(f32)
    xp = (x + bprime[None, :, None, None]).astype(f32)

    import ml_dtypes

    wgp = np.ascontiguousarray(np.concatenate([g_w, phi_w], 0).T.astype(np.float16))
    wt = np.ascontiguousarray(theta_w.T.astype(np.float16))
    wpc = np.ascontiguousarray((W_w * inv[:, None]).T.astype(ml_dtypes.bfloat16))
    bg = (g_b - g_w @ bprime).astype(f32)
    bp = (phi_b - phi_w @ bprime).astype(f32)
    bgp = np.concatenate([bg, bp]).reshape(C, 1).astype(f32)
    btv = (theta_b - theta_w @ bprime).reshape(CI, 1).astype(f32)
    id64 = np.eye(CI, dtype=np.float16)

    in_maps = []
    for core in range(8):
        s, h = core // 2, core % 2
        xs_full = np.ascontiguousarray(xp[s].reshape(C, N).astype(np.float16))
        xs_half = np.ascontiguousarray(
            xp[s, :, 48 * h:48 * (h + 1), :].reshape(C, NQH)
        )
        in_maps.append(
            {
                "xf": xs_full,
                "xh": xs_half,
                "xh16": xs_half.astype(np.float16),
                "wgp": wgp,
                "wt": wt,
                "wpc": wpc,
                "bgp": bgp,
                "bt": btv,
                "id64": id64,
            }
        )
    return in_maps


def run_cores(in_maps, trace=False):
    nc = _get_program()
    return run_bass_kernel_spmd(nc, in_maps, list(range(8)), trace=trace)


def kernel(**inputs) -> np.ndarray:
    in_maps = _host_prep(**inputs)
    res = run_cores(in_maps)
    out = np.empty((B, C, H, W), dtype=np.float32)
    for core in range(8):
        s, h = core // 2, core % 2
        out[s, :, 48 * h:48 * (h + 1), :] = res.results[core]["out"].reshape(
            C, 48, W
        )
    return out


# revision 15
# speedup vs baseline: 1.5502x; 1.2118x over previous
"""CANet non-local attention block (sparse_attention) on 8 Trainium2 cores.

Math (per sample, reference.py):
    g     = maxpool2(conv1x1(x, g_w, g_b))        -> [CI, 2304]
    theta = conv1x1(x, theta_w, theta_b)          -> [CI, 9216]
    phi   = maxpool2(conv1x1(x, phi_w, phi_b))    -> [CI, 2304]
    f     = theta^T @ phi                         -> [9216, 2304]
    attn  = softmax(f, axis=-1)
    y     = attn @ g^T                            -> [9216, CI]
    out   = BN(conv1x1(y^T, W_w, W_b)) + x

Sharding: 8 cores = 4 samples x 2 query-halves. Each core computes phi/g
over its full sample (redundantly with its pair core) and theta/attention
for its 4608-query half.

Device-side design notes:
  - All matmuls in float32r (full fp32 data, 1 cycle/row at free-dim>=256).
  - Softmax without a max pass: exp(f - 20) is safe (|f| < ~60) and the
    normalizer s = sum_k e is obtained for free by augmenting g^T with a
    ones column in the second matmul (y_aug row 64 = s).
  - BN + W_b folded into W' = inv*W_w on host; the residual-side bias b'
    is folded into x' = x + b' with projection biases compensated.
  - Final normalize: r = 1/s (DVE fast reciprocal), broadcast across
    partitions with a K=1 PE matmul against a ones row.
"""

import sys
import types

if "/opt/trn_rl_repo" not in sys.path:
    sys.path.insert(0, "/opt/trn_rl_repo")

# antenv.axon_hooks is absent in this image, so trn_boot's NTFF hook install
# silently degrades. Provide the module and install the ctypes hook ourselves
# so run_bass_kernel_spmd(trace=True) can capture NTFF profiles.
try:
    import antenv

    if "antenv.axon_hooks" not in sys.modules:
        _m = types.ModuleType("antenv.axon_hooks")
        _hook_box = [None]

        def _set(h):
            _hook_box[0] = h

        def _get():
            return _hook_box[0]

        _m.set_axon_ntff_profile_hook = _set
        _m.get_axon_ntff_profile_hook = _get
        sys.modules["antenv.axon_hooks"] = _m
        antenv.axon_hooks = _m
        try:
            if "/root/.axon_site" not in sys.path:
                sys.path.insert(0, "/root/.axon_site")
            from trn_agent_boot.trn_boot import _ntff_profile_via_ctypes

            _hook_box[0] = _ntff_profile_via_ctypes("/opt/axon/libaxon_pjrt.so")
        except Exception:
            pass
except Exception:
    pass

import numpy as np

import concourse.bass as bass
import concourse.tile as tile
from concourse import mybir
from concourse.bass_utils import run_bass_kernel_spmd

F32 = mybir.dt.float32
F32R = mybir.dt.float32r
F16 = mybir.dt.float16
BF16 = mybir.dt.bfloat16

B, C, CI, H, W = 4, 128, 64, 96, 96
N = H * W            # 9216 queries per sample
NKV = (H // 2) * (W // 2)   # 2304 keys
NQH = N // 2         # 4608 queries per core
QB = 512             # query block
NB = NQH // QB       # 9 blocks per core
KCH = NKV // 128     # 18 kv chunks of 128
EXP_SHIFT = -20.0

_PROGRAM = None  # (nc, run-callable cache)


def _r(ap):
    return ap.bitcast(F32R)


def _split_multi_waits(nc, max_waits=1):
    """walrus codegen in this container only accepts one sync-wait command
    per instruction; hoist extras onto injected same-engine NoOps."""
    n_new = 0
    for f in nc.m.functions:
        for bb in f.blocks:
            new_list = []
            for ins in bb.instructions:
                si = ins.sync_info
                w = list(si.on_wait) if si and si.on_wait else []
                if len(w) > max_waits:
                    extras, keep = w[:-max_waits], w[-max_waits:]
                    for ew in extras:
                        nop = mybir.InstNoOp(
                            name=f"I-ws{nc.next_id()}", ins=[], outs=[]
                        )
                        nop.engine = ins.engine
                        nop.sync_info = mybir.SyncInfo(on_wait=[ew], on_update=[])
                        new_list.append(nop)
                        n_new += 1
                    si.on_wait = keep
                    ins.sync_info = si
                new_list.append(ins)
            bb.instructions[:] = new_list
    return n_new


def _build_program():
    nc = bass.Bass("TRN2", target_bir_lowering=False, debug=False, num_devices=8)

    xf = nc.dram_tensor("xf", [C, N], F16, kind="ExternalInput")
    xh = nc.dram_tensor("xh", [C, NQH], F32, kind="ExternalInput")
    xh16 = nc.dram_tensor("xh16", [C, NQH], F16, kind="ExternalInput")
    wgp = nc.dram_tensor("wgp", [C, 128], F16, kind="ExternalInput")
    wt = nc.dram_tensor("wt", [C, CI], F16, kind="ExternalInput")
    wpc = nc.dram_tensor("wpc", [CI, C], BF16, kind="ExternalInput")
    bgp = nc.dram_tensor("bgp", [C, 1], F32, kind="ExternalInput")
    bt = nc.dram_tensor("bt", [CI, 1], F32, kind="ExternalInput")
    id64 = nc.dram_tensor("id64", [CI, CI], F16, kind="ExternalInput")
    out = nc.dram_tensor("out", [C, NQH], F32, kind="ExternalOutput")
    rb_dram = nc.dram_tensor("rb_scratch", [NB, QB], F32)

    with tile.TileContext(nc) as tc:
        with (
            tc.tile_pool(name="const", bufs=1) as const,
            tc.tile_pool(name="main", bufs=1) as main,
            tc.tile_pool(name="small", bufs=3) as small,
            tc.tile_pool(name="fps", bufs=2, space="PSUM") as fps,
            tc.tile_pool(name="sps", bufs=2, space="PSUM") as sps,
        ):
            wgp_sb = const.tile([C, 128], F16)
            nc.sync.dma_start(wgp_sb, wgp[:, :])
            wt_sb = const.tile([C, CI], F16)
            nc.sync.dma_start(wt_sb, wt[:, :])
            wpc_sb = const.tile([CI, C], BF16)
            nc.sync.dma_start(wpc_sb, wpc[:, :])
            bgp_sb = const.tile([C, 1], F32)
            nc.sync.dma_start(bgp_sb, bgp[:, :])
            bt_sb = const.tile([CI, 1], F32)
            nc.sync.dma_start(bt_sb, bt[:, :])
            id_sb = const.tile([CI, CI], F16)
            nc.sync.dma_start(id_sb, id64[:, :])
            shift_sb = const.tile([C, 1], F32)
            nc.vector.memset(shift_sb, EXP_SHIFT)

            xh_sb = main.tile([C, NQH], F32)
            xh16_sb = main.tile([C, NQH], F16)
            th_sb = main.tile([C, NQH], F16)
            P_sb = main.tile([C, 48, 48], F16)     # pooled [g(0:64); phi(64:128)]
            phi0 = main.tile([CI, NKV], F16)       # phi copy at base partition 0
            gt_sb = main.tile([C, KCH * (CI + 1)], BF16)  # g^T chunks + ones col
            y_all = main.tile([CI + 1, NQH], BF16)  # y rows 0..63 used
            s_coll = main.tile([NB, QB], F32)      # row b = softmax denominators
            r_coll = main.tile([NB, QB], F32)

            # ---- conv phase (gp_full lives in a pool we close afterwards) ----
            with (
                tc.tile_pool(name="big", bufs=1) as big,
                tc.tile_pool(name="xs", bufs=4) as xs,
            ):
                gp_full = big.tile([C, N], F16)
                for t in range(N // 1536):  # 6 groups of 3x512
                    ft = fps.tile([C, 1536], F32, tag="fp")
                    for u in range(3):
                        j = 3 * t + u
                        xt = xs.tile([C, QB], F16)
                        nc.sync.dma_start(xt, xf[:, j * QB:(j + 1) * QB])
                        nc.tensor.matmul(
                            ft[:, u * QB:(u + 1) * QB],
                            lhsT=wgp_sb,
                            rhs=xt,
                            start=True,
                            stop=True,
                        )
                    nc.scalar.add(
                        gp_full[:, t * 1536:(t + 1) * 1536], ft, bgp_sb
                    )
                nc.sync.dma_start(xh16_sb, xh16[:, :])
                for j in range(NB):
                    tp = sps.tile([C, QB], F32, tag="sp")
                    nc.tensor.matmul(
                        tp[0:CI, :],
                        lhsT=wt_sb,
                        rhs=xh16_sb[:, j * QB:(j + 1) * QB],
                        start=True,
                        stop=True,
                    )
                    nc.scalar.add(
                        th_sb[0:CI, j * QB:(j + 1) * QB], tp[0:CI, :], bt_sb
                    )
                    nc.scalar.add(
                        th_sb[CI:C, j * QB:(j + 1) * QB], tp[0:CI, :], bt_sb
                    )

                nc.sync.dma_start(xh_sb, xh[:, :])

                # 2x2 maxpool over the 96x96 spatial grid
                v = gp_full[:, :].rearrange(
                    "p (h a w b) -> p h a w b", h=48, a=2, w=48, b=2
                )
                m1 = big.tile([C, 48, 48], F16)
                m2 = big.tile([C, 48, 48], F16)
                nc.vector.tensor_max(m1, v[:, :, 0, :, 0], v[:, :, 0, :, 1])
                nc.vector.tensor_max(m2, v[:, :, 1, :, 0], v[:, :, 1, :, 1])
                nc.vector.tensor_max(P_sb, m1, m2)

            P_flat = P_sb[:, :, :].rearrange("p h w -> p (h w)")
            nc.vector.tensor_copy(phi0, P_flat[CI:C, :])

            # g^T chunks [128kv, 64ci] via PE transpose, plus ones column
            for j in range(KCH):
                tp = sps.tile([C, QB], F16, tag="sp")
                nc.tensor.transpose(
                    tp[:, 0:CI], P_flat[0:CI, j * 128:(j + 1) * 128], id_sb
                )
                nc.vector.tensor_copy(
                    gt_sb[:, j * (CI + 1):j * (CI + 1) + CI], tp[:, 0:CI]
                )
            onesc_sb = const.tile([C, 1], F32)
            nc.vector.memset(onesc_sb, 1.0)
            for j in range(KCH):
                nc.vector.tensor_copy(
                    gt_sb[:, j * (CI + 1) + CI:(j + 1) * (CI + 1)], onesc_sb
                )

            # ---- attention phase, software-pipelined over q-blocks ----
            with tc.tile_pool(name="epool", bufs=2) as epool:

                def emit_mm1(e_t, b, groups):
                    for t in groups:
                        ft = fps.tile([C, 1536], F32, tag="fp")
                        for u in range(3):
                            j = 3 * t + u
                            if j % 2 == 0:
                                lhsT = phi0[:, j * 128:(j + 1) * 128]
                                rhs = th_sb[0:CI, b * QB:(b + 1) * QB]
                                pos = (0, 0)
                            else:
                                lhsT = P_flat[CI:C, j * 128:(j + 1) * 128]
                                rhs = th_sb[CI:C, b * QB:(b + 1) * QB]
                                pos = (64, 0)
                            nc.tensor.matmul(
                                ft[:, u * QB:(u + 1) * QB],
                                lhsT=lhsT,
                                rhs=rhs,
                                start=True,
                                stop=True,
                                tile_position=pos,
                            )
                        nc.scalar.activation(
                            e_t[:, t * 1536:(t + 1) * 1536],
                            ft,
                            mybir.ActivationFunctionType.Exp,
                            bias=shift_sb,
                        )

                def emit_mm2(e_t, y_ps, chunks, first, last):
                    for j in chunks:
                        nc.tensor.matmul(
                            y_ps[0:CI + 1, :],
                            lhsT=gt_sb[:, j * (CI + 1):(j + 1) * (CI + 1)],
                            rhs=e_t[:, j * QB:(j + 1) * QB],
                            start=(j == first),
                            stop=(j == last),
                            skip_group_check=True,
                        )

                e_cur = epool.tile([C, N], BF16, tag="e")
                emit_mm1(e_cur, 0, range(6))
                for b in range(NB):
                    y_ps = sps.tile([C, QB], F32, tag="sp")
                    if b + 1 < NB:
                        # interleave next block's QK^T/exp with this block's AV
                        e_nxt = epool.tile([C, N], BF16, tag="e")
                        for t in range(6):
                            emit_mm2(e_cur, y_ps, range(3 * t, 3 * t + 3), 0, 17)
                            emit_mm1(e_nxt, b + 1, [t])
                    else:
                        e_nxt = None
                        emit_mm2(e_cur, y_ps, range(KCH), 0, 17)

                    nc.vector.tensor_copy(
                        y_all[0:CI, b * QB:(b + 1) * QB], y_ps[0:CI, :]
                    )
                    s_t = small.tile([1, QB], F32)
                    nc.vector.tensor_copy(s_t, y_ps[CI:CI + 1, :])
                    nc.sync.dma_start(s_coll[b:b + 1, :], s_t)
                    e_cur = e_nxt

                # one batched reciprocal for all blocks, staged to DRAM for
                # the partition-broadcast DMA
                nc.vector.reciprocal(r_coll, s_coll)
                nc.sync.dma_start(rb_dram[:, :], r_coll)

                for b in range(NB):
                    br_sb = small.tile([C, QB], F32)
                    nc.sync.dma_start(
                        br_sb, rb_dram[b:b + 1, :].partition_broadcast(C)
                    )
                    z_ps = sps.tile([C, QB], F32, tag="sp")
                    nc.tensor.matmul(
                        z_ps,
                        lhsT=wpc_sb,
                        rhs=y_all[0:CI, b * QB:(b + 1) * QB],
                        start=True,
                        stop=True,
                    )
                    t1 = small.tile([C, QB], F32)
                    nc.vector.tensor_mul(t1, z_ps, br_sb)
                    ot = small.tile([C, QB], F32)
                    nc.vector.tensor_add(ot, t1, xh_sb[:, b * QB:(b + 1) * QB])
                    nc.sync.dma_start(out[:, b * QB:(b + 1) * QB], ot)

    _split_multi_waits(nc)
    return nc


def _get_program():
    global _PROGRAM
    if _PROGRAM is None:
        _PROGRAM = _build_program()
    return _PROGRAM


def _host_prep(x, g_w, g_b, theta_w, theta_b, phi_w, phi_b, W_w, W_b,
               bn_gamma, bn_beta, bn_mean, bn_var):
    f32 = np.float32
    inv = (bn_gamma / np.sqrt(bn_var + 1e-5)).astype(f32)
    bprime = (W_b * inv + bn_beta - bn_mean * inv).astype(f32)
    xp = (x + bprime[None, :, None, None]).astype(f32)

    import ml_dtypes

    wgp = np.ascontiguousarray(np.concatenate([g_w, phi_w], 0).T.astype(np.float16))
    wt = np.ascontiguousarray(theta_w.T.astype(np.float16))
    wpc = np.ascontiguousarray((W_w * inv[:, None]).T.astype(ml_dtypes.bfloat16))
    bg = (g_b - g_w @ bprime).astype(f32)
    bp = (phi_b - phi_w @ bprime).astype(f32)
    bgp = np.concatenate([bg, bp]).reshape(C, 1).astype(f32)
    btv = (theta_b - theta_w @ bprime).reshape(CI, 1).astype(f32)
    id64 = np.eye(CI, dtype=np.float16)

    in_maps = []
    for core in range(8):
        s, h = core // 2, core % 2
        xs_full = np.ascontiguousarray(xp[s].reshape(C, N).astype(np.float16))
        xs_half = np.ascontiguousarray(
            xp[s, :, 48 * h:48 * (h + 1), :].reshape(C, NQH)
        )
        in_maps.append(
            {
                "xf": xs_full,
                "xh": xs_half,
                "xh16": xs_half.astype(np.float16),
                "wgp": wgp,
                "wt": wt,
                "wpc": wpc,
                "bgp": bgp,
                "bt": btv,
                "id64": id64,
            }
        )
    return in_maps


def run_cores(in_maps, trace=False):
    nc = _get_program()
    return run_bass_kernel_spmd(nc, in_maps, list(range(8)), trace=trace)


def kernel(**inputs) -> np.ndarray:
    in_maps = _host_prep(**inputs)
    res = run_cores(in_maps)
    out = np.empty((B, C, H, W), dtype=np.float32)
    for core in range(8):
        s, h = core // 2, core % 2
        out[s, :, 48 * h:48 * (h + 1), :] = res.results[core]["out"].reshape(
            C, 48, W
        )
    return out


# revision 16
# speedup vs baseline: 1.5845x; 1.0221x over previous
"""CANet non-local attention block (sparse_attention) on 8 Trainium2 cores.

Math (per sample, reference.py):
    g     = maxpool2(conv1x1(x, g_w, g_b))        -> [CI, 2304]
    theta = conv1x1(x, theta_w, theta_b)          -> [CI, 9216]
    phi   = maxpool2(conv1x1(x, phi_w, phi_b))    -> [CI, 2304]
    f     = theta^T @ phi                         -> [9216, 2304]
    attn  = softmax(f, axis=-1)
    y     = attn @ g^T                            -> [9216, CI]
    out   = BN(conv1x1(y^T, W_w, W_b)) + x

Sharding: 8 cores = 4 samples x 2 query-halves. Each core computes phi/g
over its full sample (redundantly with its pair core) and theta/attention
for its 4608-query half.

Device-side design notes:
  - All matmuls in float32r (full fp32 data, 1 cycle/row at free-dim>=256).
  - Softmax without a max pass: exp(f - 20) is safe (|f| < ~60) and the
    normalizer s = sum_k e is obtained for free by augmenting g^T with a
    ones column in the second matmul (y_aug row 64 = s).
  - BN + W_b folded into W' = inv*W_w on host; the residual-side bias b'
    is folded into x' = x + b' with projection biases compensated.
  - Final normalize: r = 1/s (DVE fast reciprocal), broadcast across
    partitions with a K=1 PE matmul against a ones row.
"""

import sys
import types

if "/opt/trn_rl_repo" not in sys.path:
    sys.path.insert(0, "/opt/trn_rl_repo")

# antenv.axon_hooks is absent in this image, so trn_boot's NTFF hook install
# silently degrades. Provide the module and install the ctypes hook ourselves
# so run_bass_kernel_spmd(trace=True) can capture NTFF profiles.
try:
    import antenv

    if "antenv.axon_hooks" not in sys.modules:
        _m = types.ModuleType("antenv.axon_hooks")
        _hook_box = [None]

        def _set(h):
            _hook_box[0] = h

        def _get():
            return _hook_box[0]

        _m.set_axon_ntff_profile_hook = _set
        _m.get_axon_ntff_profile_hook = _get
        sys.modules["antenv.axon_hooks"] = _m
        antenv.axon_hooks = _m
        try:
            if "/root/.axon_site" not in sys.path:
                sys.path.insert(0, "/root/.axon_site")
            from trn_agent_boot.trn_boot import _ntff_profile_via_ctypes

            _hook_box[0] = _ntff_profile_via_ctypes("/opt/axon/libaxon_pjrt.so")
        except Exception:
            pass
except Exception:
    pass

import numpy as np

import concourse.bass as bass
import concourse.tile as tile
from concourse import mybir
from concourse.bass_utils import run_bass_kernel_spmd

F32 = mybir.dt.float32
F32R = mybir.dt.float32r
F16 = mybir.dt.float16
BF16 = mybir.dt.bfloat16

B, C, CI, H, W = 4, 128, 64, 96, 96
N = H * W            # 9216 queries per sample
NKV = (H // 2) * (W // 2)   # 2304 keys
NQH = N // 2         # 4608 queries per core
QB = 512             # query block
NB = NQH // QB       # 9 blocks per core
KCH = NKV // 128     # 18 kv chunks of 128
EXP_SHIFT = -20.0

_PROGRAM = None  # (nc, run-callable cache)


def _r(ap):
    return ap.bitcast(F32R)


def _split_multi_waits(nc, max_waits=1):
    """walrus codegen in this container only accepts one sync-wait command
    per instruction; hoist extras onto injected same-engine NoOps."""
    n_new = 0
    for f in nc.m.functions:
        for bb in f.blocks:
            new_list = []
            for ins in bb.instructions:
                si = ins.sync_info
                w = list(si.on_wait) if si and si.on_wait else []
                if len(w) > max_waits:
                    extras, keep = w[:-max_waits], w[-max_waits:]
                    for ew in extras:
                        nop = mybir.InstNoOp(
                            name=f"I-ws{nc.next_id()}", ins=[], outs=[]
                        )
                        nop.engine = ins.engine
                        nop.sync_info = mybir.SyncInfo(on_wait=[ew], on_update=[])
                        new_list.append(nop)
                        n_new += 1
                    si.on_wait = keep
                    ins.sync_info = si
                new_list.append(ins)
            bb.instructions[:] = new_list
    return n_new


def _build_program():
    nc = bass.Bass("TRN2", target_bir_lowering=False, debug=False, num_devices=8)

    xf = nc.dram_tensor("xf", [C, N], F16, kind="ExternalInput")
    xh = nc.dram_tensor("xh", [C, NQH], F32, kind="ExternalInput")
    xh16 = nc.dram_tensor("xh16", [C, NQH], F16, kind="ExternalInput")
    wgp = nc.dram_tensor("wgp", [C, 128], F16, kind="ExternalInput")
    wt = nc.dram_tensor("wt", [C, CI], F16, kind="ExternalInput")
    wpc = nc.dram_tensor("wpc", [CI, C], BF16, kind="ExternalInput")
    bgp = nc.dram_tensor("bgp", [C, 1], F32, kind="ExternalInput")
    bt = nc.dram_tensor("bt", [CI, 1], F32, kind="ExternalInput")
    id64 = nc.dram_tensor("id64", [CI, CI], F16, kind="ExternalInput")
    out = nc.dram_tensor("out", [C, NQH], F32, kind="ExternalOutput")
    rb_dram = nc.dram_tensor("rb_scratch", [NB, QB], F32)

    with tile.TileContext(nc) as tc:
        with (
            tc.tile_pool(name="const", bufs=1) as const,
            tc.tile_pool(name="main", bufs=1) as main,
            tc.tile_pool(name="small", bufs=4) as small,
            tc.tile_pool(name="brp", bufs=9) as brp,
            tc.tile_pool(name="fps", bufs=2, space="PSUM") as fps,
            tc.tile_pool(name="sps", bufs=2, space="PSUM") as sps,
        ):
            wgp_sb = const.tile([C, 128], F16)
            nc.sync.dma_start(wgp_sb, wgp[:, :])
            wt_sb = const.tile([C, CI], F16)
            nc.sync.dma_start(wt_sb, wt[:, :])
            wpc_sb = const.tile([CI, C], BF16)
            nc.sync.dma_start(wpc_sb, wpc[:, :])
            bgp_sb = const.tile([C, 1], F32)
            nc.sync.dma_start(bgp_sb, bgp[:, :])
            bt_sb = const.tile([CI, 1], F32)
            nc.sync.dma_start(bt_sb, bt[:, :])
            id_sb = const.tile([CI, CI], F16)
            nc.sync.dma_start(id_sb, id64[:, :])
            shift_sb = const.tile([C, 1], F32)
            nc.vector.memset(shift_sb, EXP_SHIFT)

            xh_sb = main.tile([C, NQH], F32)
            xh16_sb = main.tile([C, NQH], F16)
            th_sb = main.tile([C, NQH], F16)
            P_sb = main.tile([C, 48, 48], F16)     # pooled [g(0:64); phi(64:128)]
            phi0 = main.tile([CI, NKV], F16)       # phi copy at base partition 0
            gt_sb = main.tile([C, KCH * (CI + 1)], BF16)  # g^T chunks + ones col
            y_all = main.tile([CI + 1, NQH], BF16)  # y rows 0..63 used
            s_coll = main.tile([NB, QB], F32)      # row b = softmax denominators
            r_coll = main.tile([NB, QB], F32)

            dummy_sb = const.tile([C, QB], F16)
            nc.vector.memset(dummy_sb, 0.0)

            def keepwarm(n):
                wp = sps.tile([C, QB], F32, tag="sp")
                for _ in range(n):
                    nc.tensor.matmul(
                        wp, lhsT=wgp_sb, rhs=dummy_sb, start=True, stop=True
                    )

            keepwarm(16)

            # ---- conv phase (gp_full lives in a pool we close afterwards) ----
            with (
                tc.tile_pool(name="big", bufs=1) as big,
                tc.tile_pool(name="xs", bufs=4) as xs,
            ):
                gp_full = big.tile([C, N], F16)
                for t in range(N // 1536):  # 6 groups of 3x512
                    ft = fps.tile([C, 1536], F32, tag="fp")
                    for u in range(3):
                        j = 3 * t + u
                        xt = xs.tile([C, QB], F16)
                        nc.sync.dma_start(xt, xf[:, j * QB:(j + 1) * QB])
                        nc.tensor.matmul(
                            ft[:, u * QB:(u + 1) * QB],
                            lhsT=wgp_sb,
                            rhs=xt,
                            start=True,
                            stop=True,
                        )
                    nc.scalar.add(
                        gp_full[:, t * 1536:(t + 1) * 1536], ft, bgp_sb
                    )
                nc.sync.dma_start(xh16_sb, xh16[:, :])
                for j in range(NB):
                    tp = sps.tile([C, QB], F32, tag="sp")
                    nc.tensor.matmul(
                        tp[0:CI, :],
                        lhsT=wt_sb,
                        rhs=xh16_sb[:, j * QB:(j + 1) * QB],
                        start=True,
                        stop=True,
                    )
                    nc.scalar.add(
                        th_sb[0:CI, j * QB:(j + 1) * QB], tp[0:CI, :], bt_sb
                    )
                    nc.scalar.add(
                        th_sb[CI:C, j * QB:(j + 1) * QB], tp[0:CI, :], bt_sb
                    )

                nc.sync.dma_start(xh_sb, xh[:, :])
                keepwarm(14)

                # 2x2 maxpool over the 96x96 spatial grid
                v = gp_full[:, :].rearrange(
                    "p (h a w b) -> p h a w b", h=48, a=2, w=48, b=2
                )
                m1 = big.tile([C, 48, 48], F16)
                m2 = big.tile([C, 48, 48], F16)
                nc.vector.tensor_max(m1, v[:, :, 0, :, 0], v[:, :, 0, :, 1])
                nc.vector.tensor_max(m2, v[:, :, 1, :, 0], v[:, :, 1, :, 1])
                nc.vector.tensor_max(P_sb, m1, m2)

            P_flat = P_sb[:, :, :].rearrange("p h w -> p (h w)")
            nc.vector.tensor_copy(phi0, P_flat[CI:C, :])

            # g^T chunks [128kv, 64ci] via PE transpose, plus ones column
            for j in range(KCH):
                tp = sps.tile([C, QB], F16, tag="sp")
                nc.tensor.transpose(
                    tp[:, 0:CI], P_flat[0:CI, j * 128:(j + 1) * 128], id_sb
                )
                nc.vector.tensor_copy(
                    gt_sb[:, j * (CI + 1):j * (CI + 1) + CI], tp[:, 0:CI]
                )
            onesc_sb = const.tile([C, 1], F32)
            nc.vector.memset(onesc_sb, 1.0)
            for j in range(KCH):
                nc.vector.tensor_copy(
                    gt_sb[:, j * (CI + 1) + CI:(j + 1) * (CI + 1)], onesc_sb
                )

            # ---- attention phase, software-pipelined over q-blocks ----
            with tc.tile_pool(name="epool", bufs=2) as epool:

                def emit_mm1(e_t, b, groups):
                    for t in groups:
                        ft = fps.tile([C, 1536], F32, tag="fp")
                        for u in range(3):
                            j = 3 * t + u
                            if j % 2 == 0:
                                lhsT = phi0[:, j * 128:(j + 1) * 128]
                                rhs = th_sb[0:CI, b * QB:(b + 1) * QB]
                                pos = (0, 0)
                            else:
                                lhsT = P_flat[CI:C, j * 128:(j + 1) * 128]
                                rhs = th_sb[CI:C, b * QB:(b + 1) * QB]
                                pos = (64, 0)
                            nc.tensor.matmul(
                                ft[:, u * QB:(u + 1) * QB],
                                lhsT=lhsT,
                                rhs=rhs,
                                start=True,
                                stop=True,
                                tile_position=pos,
                            )
                        nc.scalar.activation(
                            e_t[:, t * 1536:(t + 1) * 1536],
                            ft,
                            mybir.ActivationFunctionType.Exp,
                            bias=shift_sb,
                        )

                def emit_mm2(e_t, y_ps, chunks, first, last):
                    for j in chunks:
                        nc.tensor.matmul(
                            y_ps[0:CI + 1, :],
                            lhsT=gt_sb[:, j * (CI + 1):(j + 1) * (CI + 1)],
                            rhs=e_t[:, j * QB:(j + 1) * QB],
                            start=(j == first),
                            stop=(j == last),
                            skip_group_check=True,
                        )

                e_cur = epool.tile([C, N], BF16, tag="e")
                emit_mm1(e_cur, 0, range(6))
                for b in range(NB):
                    y_ps = sps.tile([C, QB], F32, tag="sp")
                    if b + 1 < NB:
                        # interleave next block's QK^T/exp with this block's AV
                        e_nxt = epool.tile([C, N], BF16, tag="e")
                        for t in range(6):
                            emit_mm2(e_cur, y_ps, range(3 * t, 3 * t + 3), 0, 17)
                            emit_mm1(e_nxt, b + 1, [t])
                    else:
                        e_nxt = None
                        emit_mm2(e_cur, y_ps, range(KCH), 0, 17)

                    nc.vector.tensor_copy(
                        y_all[0:CI, b * QB:(b + 1) * QB], y_ps[0:CI, :]
                    )
                    s_t = small.tile([1, QB], F32)
                    nc.vector.tensor_copy(s_t, y_ps[CI:CI + 1, :])
                    nc.sync.dma_start(s_coll[b:b + 1, :], s_t)
                    if b == NB - 2:
                        # blocks 0..7: batched reciprocal, overlaps block 8
                        nc.vector.reciprocal(
                            r_coll[0:NB - 1, :], s_coll[0:NB - 1, :]
                        )
                        nc.sync.dma_start(
                            rb_dram[0:NB - 1, :], r_coll[0:NB - 1, :]
                        )
                    e_cur = e_nxt

                # block 8's reciprocal (partition-8 start is not DVE-legal;
                # hop through partition 0 via DMA)
                s8 = small.tile([1, QB], F32)
                nc.sync.dma_start(s8, s_coll[NB - 1:NB, :])
                r8 = small.tile([1, QB], F32)
                nc.vector.reciprocal(r8, s8)
                nc.sync.dma_start(rb_dram[NB - 1:NB, :], r8)

                for b in range(NB):
                    br_sb = brp.tile([C, QB], F32)
                    nc.sync.dma_start(
                        br_sb, rb_dram[b:b + 1, :].partition_broadcast(C)
                    )
                    z_ps = sps.tile([C, QB], F32, tag="sp")
                    nc.tensor.matmul(
                        z_ps,
                        lhsT=wpc_sb,
                        rhs=y_all[0:CI, b * QB:(b + 1) * QB],
                        start=True,
                        stop=True,
                    )
                    t1 = small.tile([C, QB], F32)
                    nc.vector.tensor_mul(t1, z_ps, br_sb)
                    ot = small.tile([C, QB], F32)
                    nc.vector.tensor_add(ot, t1, xh_sb[:, b * QB:(b + 1) * QB])
                    nc.sync.dma_start(out[:, b * QB:(b + 1) * QB], ot)

    _split_multi_waits(nc)
    return nc


def _get_program():
    global _PROGRAM
    if _PROGRAM is None:
        _PROGRAM = _build_program()
    return _PROGRAM


def _host_prep(x, g_w, g_b, theta_w, theta_b, phi_w, phi_b, W_w, W_b,
               bn_gamma, bn_beta, bn_mean, bn_var):
    f32 = np.float32
    inv = (bn_gamma / np.sqrt(bn_var + 1e-5)).astype(f32)
    bprime = (W_b * inv + bn_beta - bn_mean * inv).astype(f32)
    xp = (x + bprime[None, :, None, None]).astype(f32)

    import ml_dtypes

    wgp = np.ascontiguousarray(np.concatenate([g_w, phi_w], 0).T.astype(np.float16))
    wt = np.ascontiguousarray(theta_w.T.astype(np.float16))
    wpc = np.ascontiguousarray((W_w * inv[:, None]).T.astype(ml_dtypes.bfloat16))
    bg = (g_b - g_w @ bprime).astype(f32)
    bp = (phi_b - phi_w @ bprime).astype(f32)
    bgp = np.concatenate([bg, bp]).reshape(C, 1).astype(f32)
    btv = (theta_b - theta_w @ bprime).reshape(CI, 1).astype(f32)
    id64 = np.eye(CI, dtype=np.float16)

    in_maps = []
    for core in range(8):
        s, h = core // 2, core % 2
        xs_full = np.ascontiguousarray(xp[s].reshape(C, N).astype(np.float16))
        xs_half = np.ascontiguousarray(
            xp[s, :, 48 * h:48 * (h + 1), :].reshape(C, NQH)
        )
        in_maps.append(
            {
                "xf": xs_full,
                "xh": xs_half,
                "xh16": xs_half.astype(np.float16),
                "wgp": wgp,
                "wt": wt,
                "wpc": wpc,
                "bgp": bgp,
                "bt": btv,
                "id64": id64,
            }
        )
    return in_maps


def run_cores(in_maps, trace=False):
    nc = _get_program()
    return run_bass_kernel_spmd(nc, in_maps, list(range(8)), trace=trace)


def kernel(**inputs) -> np.ndarray:
    in_maps = _host_prep(**inputs)
    res = run_cores(in_maps)
    out = np.empty((B, C, H, W), dtype=np.float32)
    for core in range(8):
        s, h = core // 2, core % 2
        out[s, :, 48 * h:48 * (h + 1), :] = res.results[core]["out"].reshape(
            C, 48, W
        )
    return out
